# revision 1
# baseline (speedup 1.0000x reference)
"""Chamfer distance kernel for Trainium2 (8 NeuronCores, data-parallel over batch).

Math (per batch b):
  p = pred / max(||pred||, eps);  t likewise (unit vectors)
  d2[n,m] = |p_n - t_m|^2 = 2 - 2*p_n.t_m   (+ O(1e-7) norm corrections, dropped)
  chamfer_b = sum_n sqrt(max(min_m d2, 0)) + sum_m sqrt(max(min_n d2, 0))
  output = mean_b chamfer_b

Device strategy (per core, 2 batches):
  * Normalization + fp16 hi/lo split computed on device in [128, x] layouts.
  * d2 via PE matmuls with augmented K=10 fp16 contraction:
      lhsT rows = [-2*hi(3), -2*hi(3), -2*lo(3), 2.0]
      rhs  rows = [ hi(3),   lo(3),    hi(3),   1.0]
    giving d2 = 2 - 2*(hi.hi' + hi.lo' + lo.hi') in f32 PSUM (err ~1e-6).
    Both matrix orientations are computed (row-mins only ever needed).
  * Min-reduction per 128-row x 2048-col PSUM half-strip:
      ACT copies cols 1024:2048 to SBUF, DVE tensor_tensor_reduce(min,min)
      consumes (psum cols 0:1024, sbuf copy) at 2 elems/lane/cycle.
  * Tails: sqrt(relu(min_d2)), row-sum; host sums the final 128-vectors.
"""
import sys
import numpy as np

sys.path.insert(0, "/opt/trn_rl_repo")

import concourse.bass as bass  # noqa: E402
import concourse.bacc as bacc  # noqa: E402
import concourse.tile as tile  # noqa: E402
import concourse.mybir as mybir  # noqa: E402
from concourse.bass_utils import run_bass_kernel_spmd  # noqa: E402

F32 = mybir.dt.float32
F16 = mybir.dt.float16
MIN = mybir.AluOpType.min

B, N, D = 16, 4096, 3
NCORES = 8
BPC = B // NCORES          # batches per core
EPS = 1e-12
NCHUNK = N // 128          # 32 row chunks per direction
HALF = 2048                # psum half-strip width (4 banks)
XSPL = HALF // 2           # reduce split: DVE stream / ACT copy width


def _build_kernel(tc, out_ap, ins):
    nc = tc.nc
    with (
        tc.tile_pool(name="prep", bufs=1) as prep,
        tc.tile_pool(name="small", bufs=8) as small,
        tc.tile_pool(name="stage", bufs=5) as stage,
        tc.tile_pool(name="oper", bufs=1) as oper,
        tc.tile_pool(name="work", bufs=3) as work,
        tc.tile_pool(name="scr", bufs=2) as scr,
        tc.tile_pool(name="col", bufs=1) as col,
        tc.tile_pool(name="psum", bufs=2, space="PSUM") as pp,
    ):
        # ---------------- prep + operand assembly ----------------
        # All per-point math in [128, x] point-major layouts; planar fp16
        # coord planes are flattened to coord-major [1, N] rows via DMA
        # (dst[32p + j] <- src[p, j]); then A/B operand tiles are assembled.
        A = {}
        Bm = {}
        for b in range(BPC):
            for s, nm in ((0, "p"), (1, "t")):
                pm_name = "pred_pm" if s == 0 else "target_pm"
                pm = small.tile([128, 96], F32, tag="pm")
                nc.sync.dma_start(out=pm[:], in_=ins[pm_name][b])
                pm3 = pm[:].rearrange("p (j d) -> p j d", d=3)
                sq = small.tile([128, 96], F32, tag="sq")
                nc.vector.tensor_mul(sq[:], pm[:], pm[:])
                sq3 = sq[:].rearrange("p (j d) -> p j d", d=3)
                pn = small.tile([128, 32], F32, tag="pn")
                nc.vector.tensor_add(pn[:], sq3[:, :, 0], sq3[:, :, 1])
                pn2 = small.tile([128, 32], F32, tag="pn2")
                nc.vector.tensor_add(pn2[:], pn[:], sq3[:, :, 2])
                nrm = small.tile([128, 32], F32, tag="nrm")
                nc.scalar.sqrt(nrm[:], pn2[:])
                nrm2 = small.tile([128, 32], F32, tag="nrm2")
                nc.vector.tensor_scalar_max(nrm2[:], nrm[:], EPS)
                inv = small.tile([128, 32], F32, tag="inv")
                nc.vector.reciprocal(inv[:], nrm2[:])
                # normalized coords, planar: plane k = cols [32k, 32k+32)
                pnr = small.tile([128, 96], F32, tag="pnr")
                for k in range(3):
                    nc.vector.tensor_mul(
                        pnr[:, 32 * k : 32 * (k + 1)], pm3[:, :, k], inv[:]
                    )
                hi = small.tile([128, 96], F16, tag="hi")
                nc.scalar.copy(hi[:], pnr[:])
                hiF = small.tile([128, 96], F32, tag="hiF")
                nc.scalar.copy(hiF[:], hi[:])
                lo = small.tile([128, 96], F16, tag="lo")
                nc.vector.tensor_sub(lo[:], pnr[:], hiF[:])
                m2hi = small.tile([128, 96], F16, tag="m2hi")
                nc.vector.tensor_scalar_mul(m2hi[:], hi[:], -2.0)
                m2lo = small.tile([128, 96], F16, tag="m2lo")
                nc.vector.tensor_scalar_mul(m2lo[:], lo[:], -2.0)
                # flatten planes to coord-major staging rows
                stg = {}
                for src, key in ((hi, "hi"), (lo, "lo"), (m2hi, "m2hi"), (m2lo, "m2lo")):
                    cm = stage.tile([3, N], F16, tag="cm")
                    for k in range(3):
                        nc.sync.dma_start(
                            out=cm[k : k + 1, :], in_=src[:, 32 * k : 32 * (k + 1)]
                        )
                    stg[key] = cm
                # operand tiles
                a = oper.tile([10, N], F16, tag=f"A{nm}{b}")
                nc.sync.dma_start(out=a[0:3, :], in_=stg["m2hi"][:])
                nc.sync.dma_start(out=a[3:6, :], in_=stg["m2hi"][:])
                nc.sync.dma_start(out=a[6:9, :], in_=stg["m2lo"][:])
                nc.sync.dma_start(out=a[9:10, :], in_=ins["consts"][0:1])
                bb = oper.tile([10, N], F16, tag=f"B{nm}{b}")
                nc.sync.dma_start(out=bb[0:3, :], in_=stg["hi"][:])
                nc.sync.dma_start(out=bb[3:6, :], in_=stg["lo"][:])
                nc.sync.dma_start(out=bb[6:9, :], in_=stg["hi"][:])
                nc.sync.dma_start(out=bb[9:10, :], in_=ins["consts"][1:2])
                A[(nm, b)] = a
                Bm[(nm, b)] = bb

        # ---------------- main loop ----------------
        # Per strip (row-chunk i of one direction): two psum halves, each
        # reduced by a chained tensor_tensor_scan(min,min) over (psum half,
        # ACT-copied sbuf half).  The second scan's out tile lives in a ring
        # so that per-strip minima can be gathered GRP at a time with one
        # strided copy.
        GRP = 4
        for b in range(BPC):
            for d in range(2):
                if (b, d) != (0, 0):
                    # Full-sync rendezvous between sections: without it, the
                    # full-size straight-line pipeline wedges the exec unit
                    # (scale-dependent hang; see NCHUNK bisect).
                    tc.strict_bb_all_engine_barrier()
                lhs = A[("p", b)] if d == 0 else A[("t", b)]
                rhs = Bm[("t", b)] if d == 0 else Bm[("p", b)]
                md = col.tile([128, NCHUNK], F32, tag=f"md{b}{d}")
                ring = None
                for i in range(NCHUNK):
                    lhs_i = lhs[:, 128 * i : 128 * (i + 1)]
                    g = i % GRP
                    if g == 0:
                        ring = scr.tile([128, GRP * XSPL], F32, tag="ring")
                    so_a = None
                    for h in range(2):
                        pt = pp.tile([128, HALF], F32, tag="pt")
                        for q in range(4):
                            c0 = HALF * h + 512 * q
                            nc.tensor.matmul(
                                pt[:, 512 * q : 512 * (q + 1)],
                                lhs_i,
                                rhs[:, c0 : c0 + 512],
                                start=True,
                                stop=True,
                            )
                        cp = work.tile([128, XSPL], F32, tag="cp")
                        nc.scalar.copy(cp[:], pt[:, XSPL:HALF])
                        if h == 0:
                            soa_t = work.tile([128, XSPL], F32, tag="soA")
                            so = soa_t[:]
                            init = 1e30
                        else:
                            so = ring[:, XSPL * g : XSPL * (g + 1)]
                            init = so_a[:, XSPL - 1 : XSPL]
                        nc.vector.tensor_tensor_scan(
                            so, pt[:, 0:XSPL], cp[:], init, MIN, MIN
                        )
                        if h == 0:
                            so_a = so
                    if g == GRP - 1:
                        # gather the GRP strip-minima (last scan columns)
                        rv = ring[:].rearrange("p (g x) -> p g x", g=GRP)
                        nc.vector.tensor_copy(
                            md[:, i - GRP + 1 : i + 1], rv[:, :, XSPL - 1]
                        )
                # tail: sqrt(relu(min_d2)), row-sum
                mdr = col.tile([128, NCHUNK], F32, tag="mdr")
                nc.vector.tensor_scalar_max(mdr[:], md[:], 0.0)
                dst = col.tile([128, NCHUNK], F32, tag="dst")
                nc.scalar.sqrt(dst[:], mdr[:])
                sm = col.tile([128, 1], F32, tag="sm")
                nc.vector.reduce_sum(sm[:], dst[:], axis=mybir.AxisListType.X)
                nc.sync.dma_start(out=out_ap[b, d], in_=sm[:])


_CACHE = {}


def _get_program():
    if "nc" not in _CACHE:
        nc = bacc.Bacc("TRN2", target_bir_lowering=False, debug=False)
        ins = {
            "pred_pm": nc.dram_tensor("pred_pm", [BPC, N, D], F32, kind="ExternalInput").ap(),
            "target_pm": nc.dram_tensor("target_pm", [BPC, N, D], F32, kind="ExternalInput").ap(),
            "consts": nc.dram_tensor("consts", [2, N], F16, kind="ExternalInput").ap(),
        }
        out = nc.dram_tensor("out", [BPC, 2, 128], F32, kind="ExternalOutput").ap()
        with tile.TileContext(nc) as tc:
            _build_kernel(tc, out, ins)
        nc.compile()
        _CACHE["nc"] = nc
    return _CACHE["nc"]


def kernel(pred: np.ndarray, target: np.ndarray, _return_results: bool = False):
    pred = np.ascontiguousarray(np.asarray(pred, dtype=np.float32))
    target = np.ascontiguousarray(np.asarray(target, dtype=np.float32))
    assert pred.shape == (B, N, D) and target.shape == (B, N, D)

    nc = _get_program()
    consts = np.empty((2, N), np.float16)
    consts[0] = 2.0
    consts[1] = 1.0
    in_maps = []
    for c in range(NCORES):
        lo, hi = c * BPC, (c + 1) * BPC
        p, t = pred[lo:hi], target[lo:hi]
        in_maps.append(
            {
                "pred_pm": np.ascontiguousarray(p),
                "target_pm": np.ascontiguousarray(t),
                "consts": consts,
            }
        )
    res = run_bass_kernel_spmd(nc, in_maps, list(range(NCORES)))
    total = 0.0
    for c in range(NCORES):
        total += float(res.results[c]["out"].astype(np.float64).sum())
    val = np.float32(total / B)
    if _return_results:
        return np.asarray(val), res
    return np.asarray(val)


if __name__ == "__main__":
    rng = np.random.default_rng(0)
    p = rng.standard_normal((B, N, D), dtype=np.float32)
    t = rng.standard_normal((B, N, D), dtype=np.float32)
    print("kernel output:", kernel(p, t))

